# revision 7
# baseline (speedup 1.0000x reference)
"""LFMA adapter kernel for 8 Trainium2 NeuronCores.

y = x @ W_base.T + b + alpha * x @ Re(ifft2(scatter(c)))      x:[2,64,4096]

With A = B = IDFT_4096 matrix (A[m,k] = exp(2pi i m k/4096)/4096) and F the
dense complex [4096,4096] scatter of c at mask_idx:

    y2 = alpha * Re( (x @ A) @ F @ B )

x is real, so u = x@A is Hermitian along k (u[N-k] = conj(u[k])): only
k = 0..2048 is needed (padded to KH=2176 = 17 tiles). The F contraction
folds into P[k] = F[k]+F[N-k], M[k] = F[k]-F[N-k]:
    vr = ur@Pr - ui@Mi,  vi = ur@Pi + ui@Mr
Sharding (8 cores): stage 1 u columns (272/core, all-gather uT), stage 2
P/M columns (512/core, all-gather vT), stage 3 output columns (512/core).
bf16 matmuls, fp32 PSUM accumulation; the y2 path is ~1e-3 of the output,
so its bf16 error is negligible; base matmul bf16 gives ~2e-3 absmax rel.
"""

import numpy as np
import ml_dtypes

import concourse.bass as bass
import concourse.mybir as mybir
import concourse.tile as tile
from concourse import bacc
from concourse.bass import ts
from concourse.bass_utils import run_bass_kernel_spmd
from concourse.masks import make_identity

BF16 = mybir.dt.bfloat16
F32 = mybir.dt.float32
NP_BF16 = ml_dtypes.bfloat16

D = 4096          # d1 == d2
T = 128           # 2*64 flattened tokens
NCORES = 8
SH = D // NCORES  # 512 output columns per core
NT = D // 128     # 32 contraction tiles over d / l
KH = 2176         # half-spectrum k range 0..2048 padded to 17*128
KT = KH // 128    # 17 k tiles
SHK = KH // NCORES  # 272 u-columns per core
ALPHA = 16.0

_CACHE = {}


def _tilemaj(m):
    """[128*nt, n] -> tile-major [128, nt*n] bf16 (tile i at cols i*n:(i+1)*n)."""
    rows, n = m.shape
    nt = rows // 128
    return np.ascontiguousarray(
        m.reshape(nt, 128, n).transpose(1, 0, 2).reshape(128, nt * n)
    ).astype(NP_BF16)


def _build_program(reps=1):
    nc = bacc.Bacc("TRN2", target_bir_lowering=False, debug=False,
                   num_devices=NCORES)
    BIG = [128, NT * SH]
    PMS = [128, KT * SH]
    AS = [128, NT * SHK]

    xt = nc.dram_tensor("xt", [128, NT * 128], BF16, kind="ExternalInput")
    a_r = nc.dram_tensor("a_r", AS, BF16, kind="ExternalInput")
    a_i = nc.dram_tensor("a_i", AS, BF16, kind="ExternalInput")
    p_r = nc.dram_tensor("p_r", PMS, BF16, kind="ExternalInput")
    p_i = nc.dram_tensor("p_i", PMS, BF16, kind="ExternalInput")
    m_r = nc.dram_tensor("m_r", PMS, BF16, kind="ExternalInput")
    m_i = nc.dram_tensor("m_i", PMS, BF16, kind="ExternalInput")
    b_r = nc.dram_tensor("b_r", BIG, BF16, kind="ExternalInput")
    b_ni = nc.dram_tensor("b_ni", BIG, BF16, kind="ExternalInput")
    w_t = nc.dram_tensor("w_t", BIG, BF16, kind="ExternalInput")
    bias = nc.dram_tensor("bias", [1, SH], BF16, kind="ExternalInput")
    y_out = nc.dram_tensor("y", [T, SH], F32, kind="ExternalOutput")

    RG = [list(range(NCORES))]

    with tile.TileContext(nc) as tc:
        with (
            tc.tile_pool(name="dram", bufs=1, space="DRAM") as dramp,
            tc.tile_pool(name="const", bufs=1) as constp,
            tc.tile_pool(name="wpool", bufs=4) as wpool,
            tc.tile_pool(name="work", bufs=2) as work,
            tc.tile_pool(name="acc", bufs=1, space="PSUM") as accp,
            tc.tile_pool(name="trp", bufs=4, space="PSUM") as trp,
        ):
            ident = constp.tile([128, 128], BF16, name="ident")
            make_identity(nc, ident)
            ones = constp.tile([1, 128], BF16, name="ones")
            nc.vector.memset(ones, 1.0)
            bias_sb = constp.tile([1, SH], BF16, name="bias_sb")
            nc.sync.dma_start(bias_sb, bias[:])

            xt_sb = constp.tile([128, NT * 128], BF16, name="xt_sb")
            nc.sync.dma_start(xt_sb, xt[:])
            xt_v = xt_sb.rearrange("p (i c) -> p i c", i=NT)

            ut_all = constp.tile([128, KT * 256], BF16, name="ut_all")
            vt_all = constp.tile([128, NT * 256], BF16, name="vt_all")
            ut_v = ut_all.rearrange("p (i c) -> p i c", i=KT)
            vt_v = vt_all.rearrange("p (i c) -> p i c", i=NT)

            for _rep in range(reps):
                ut_sh = dramp.tile([SHK, 2, 128], BF16, tag="ut_sh",
                                   name=f"ut_sh{_rep}")
                ut_ag = dramp.tile([KH, 2, 128], BF16, tag="ut_ag",
                                   name=f"ut_ag{_rep}", addr_space="Shared")
                vt_sh = dramp.tile([4, 128, 256], BF16, tag="vt_sh",
                                   name=f"vt_sh{_rep}")
                vt_ag = dramp.tile([NT, 128, 256], BF16, tag="vt_ag",
                                   name=f"vt_ag{_rep}", addr_space="Shared")

                def load_big(src, shape, nm):
                    t = wpool.tile(shape, BF16, tag="big", name=nm)
                    nc.gpsimd.dma_start(t, src[:])
                    return t

                ar_sb = load_big(a_r, AS, f"ar_sb{_rep}")
                ai_sb = load_big(a_i, AS, f"ai_sb{_rep}")
                pr_sb = load_big(p_r, PMS, f"pr_sb{_rep}")
                mi_sb = load_big(m_i, PMS, f"mi_sb{_rep}")
                pi_sb = load_big(p_i, PMS, f"pi_sb{_rep}")
                mr_sb = load_big(m_r, PMS, f"mr_sb{_rep}")

                # ---- stage 1: u = x @ A_shard   [128, 272] per component ----
                ps_ur = accp.tile([T, SHK], F32, tag="s1a", name=f"ps_ur{_rep}")
                ps_ui = accp.tile([T, SHK], F32, tag="s1b", name=f"ps_ui{_rep}")
                for i in range(NT):
                    nc.tensor.matmul(ps_ur, xt_v[:, i], ar_sb[:, ts(i, SHK)],
                                     start=(i == 0), stop=(i == NT - 1))
                for i in range(NT):
                    nc.tensor.matmul(ps_ui, xt_v[:, i], ai_sb[:, ts(i, SHK)],
                                     start=(i == 0), stop=(i == NT - 1))

                ur_sb = work.tile([T, SHK], BF16, tag="uv", name=f"ur{_rep}")
                nc.vector.tensor_copy(out=ur_sb, in_=ps_ur)
                ui_sb = work.tile([T, SHK], BF16, tag="uv", name=f"ui{_rep}")
                nc.vector.tensor_copy(out=ui_sb, in_=ps_ui)
                # transpose 272 = 2x128 + 16 into [k, comp, t] staging
                stg = work.tile([128, 3 * 256], BF16, tag="stg",
                                name=f"ustg{_rep}")
                for j in range(3):
                    rows = 128 if j < 2 else 16
                    for h, comp in enumerate((ur_sb, ui_sb)):
                        pt = trp.tile([128, 128], BF16, tag="tr",
                                      name=f"utp{_rep}_{j}_{h}")
                        nc.tensor.transpose(
                            pt[:rows, :], comp[:, j * 128: j * 128 + rows],
                            ident)
                        nc.vector.tensor_copy(
                            out=stg[:rows, j * 256 + h * 128:
                                    j * 256 + h * 128 + 128],
                            in_=pt[:rows, :])
                nc.sync.dma_start(
                    out=ut_sh[0:256].rearrange("(j p) h c -> p j h c", p=128),
                    in_=stg[:, 0:512].rearrange("p (j h c) -> p j h c",
                                                j=2, h=2))
                nc.sync.dma_start(
                    out=ut_sh[256:272],
                    in_=stg[0:16, 512:768].rearrange("p (h c) -> p h c", h=2))

                nc.gpsimd.collective_compute(
                    "AllGather", mybir.AluOpType.bypass,
                    ins=[ut_sh.opt()], outs=[ut_ag.opt()], replica_groups=RG)

                nc.sync.dma_start(
                    out=ut_all.rearrange("p (i c) -> p i c", i=KT),
                    in_=ut_ag.rearrange("(i p) h c -> p i (h c)", p=128))

                # ---- stage 2: vr = ur@Pr - ui@Mi ; vi = ur@Pi + ui@Mr ----
                ps_a = accp.tile([T, SH], F32, tag="s1a", name=f"ps_a{_rep}")
                ps_b = accp.tile([T, SH], F32, tag="s1b", name=f"ps_b{_rep}")
                ps_c = accp.tile([T, SH], F32, tag="s2c", name=f"ps_c{_rep}")
                for i in range(KT):
                    nc.tensor.matmul(ps_a, ut_v[:, i, 0:128],
                                     pr_sb[:, ts(i, SH)],
                                     start=(i == 0), stop=(i == KT - 1))
                for i in range(KT):
                    nc.tensor.matmul(ps_b, ut_v[:, i, 128:256],
                                     mi_sb[:, ts(i, SH)],
                                     start=(i == 0), stop=(i == KT - 1))
                for i in range(KT):
                    nc.tensor.matmul(ps_c, ut_v[:, i, 0:128],
                                     pi_sb[:, ts(i, SH)],
                                     start=(i == 0), stop=False)
                for i in range(KT):
                    nc.tensor.matmul(ps_c, ut_v[:, i, 128:256],
                                     mr_sb[:, ts(i, SH)],
                                     start=False, stop=(i == KT - 1))
                tmp_a = work.tile([T, SH], F32, tag="tmpf", name=f"tmpa{_rep}")
                nc.vector.tensor_copy(out=tmp_a, in_=ps_a)
                vr_sb = work.tile([T, SH], BF16, tag="uv2", name=f"vr{_rep}")
                nc.vector.tensor_sub(out=vr_sb, in0=tmp_a, in1=ps_b)
                vi_sb = work.tile([T, SH], BF16, tag="uv2", name=f"vi{_rep}")
                nc.vector.tensor_copy(out=vi_sb, in_=ps_c)

                br_sb = load_big(b_r, BIG, f"br_sb{_rep}")
                bni_sb = load_big(b_ni, BIG, f"bni_sb{_rep}")

                stg2 = work.tile([128, 4 * 256], BF16, tag="stg",
                                 name=f"vstg{_rep}")
                for j in range(4):
                    for h, comp in enumerate((vr_sb, vi_sb)):
                        pt = trp.tile([128, 128], BF16, tag="tr",
                                      name=f"vtp{_rep}_{j}_{h}")
                        nc.tensor.transpose(pt, comp[:, ts(j, 128)], ident)
                        nc.vector.tensor_copy(
                            out=stg2[:, j * 256 + h * 128:
                                     j * 256 + (h + 1) * 128],
                            in_=pt)
                nc.sync.dma_start(
                    out=vt_sh.rearrange("j p c -> p j c"),
                    in_=stg2.rearrange("p (j c) -> p j c", j=4))

                nc.gpsimd.collective_compute(
                    "AllGather", mybir.AluOpType.bypass,
                    ins=[vt_sh.opt()], outs=[vt_ag.opt()], replica_groups=RG)

                nc.sync.dma_start(
                    out=vt_all.rearrange("p (i c) -> p i c", i=NT),
                    in_=vt_ag.rearrange("i p c -> p i c"))

                wt_sb = load_big(w_t, BIG, f"wt_sb{_rep}")

                # ---- stage 3: y = vTr.T@(aBr) + vTi.T@(-aBi) + x@WbT + b ----
                ps_y = accp.tile([T, SH], F32, tag="s3", name=f"ps_y{_rep}")
                for i in range(NT):
                    nc.tensor.matmul(ps_y, vt_v[:, i, 0:128],
                                     br_sb[:, ts(i, SH)],
                                     start=(i == 0), stop=False)
                for i in range(NT):
                    nc.tensor.matmul(ps_y, vt_v[:, i, 128:256],
                                     bni_sb[:, ts(i, SH)],
                                     start=False, stop=False)
                for i in range(NT):
                    nc.tensor.matmul(ps_y, xt_v[:, i], wt_sb[:, ts(i, SH)],
                                     start=False, stop=False)
                nc.tensor.matmul(ps_y, ones, bias_sb, start=False, stop=True)

                y_sb = work.tile([T, SH], F32, tag="ysb", name=f"y_sb{_rep}")
                nc.vector.tensor_copy(out=y_sb, in_=ps_y)
                nc.sync.dma_start(out=y_out[:], in_=y_sb)

    nc.compile()
    return nc


def _host_prep(x, W_base, b_base, c_re, c_im, mask_idx):
    xf = np.asarray(x, np.float32).reshape(T, D)
    xT = _tilemaj(np.ascontiguousarray(xf.T))

    # exact-phase IDFT matrix via cos/sin table lookup
    idx = np.arange(D, dtype=np.int64)
    tab_c = (np.cos(2 * np.pi * np.arange(D) / D) / D).astype(np.float32)
    tab_s = (np.sin(2 * np.pi * np.arange(D) / D) / D).astype(np.float32)
    ph_half = (idx[:, None] * idx[None, :KH]) % D     # [4096, 2176]
    Ar = tab_c[ph_half]
    Ai = tab_s[ph_half]
    Ar[:, 2049:] = 0.0
    Ai[:, 2049:] = 0.0
    del ph_half

    Fr = np.zeros(D * D, np.float32)
    Fi = np.zeros(D * D, np.float32)
    mi = np.asarray(mask_idx, np.int64)
    Fr[mi] = np.asarray(c_re, np.float32)
    Fi[mi] = np.asarray(c_im, np.float32)
    Fr = Fr.reshape(D, D)
    Fi = Fi.reshape(D, D)
    rev = (-np.arange(KH)) % D
    Pr = Fr[:KH] + Fr[rev]
    Pi = Fi[:KH] + Fi[rev]
    Mr = Fr[:KH] - Fr[rev]
    Mi = Fi[:KH] - Fi[rev]
    for M_ in (Pr, Pi, Mr, Mi):
        M_[0] *= 0.5
        M_[2048] *= 0.5
        M_[2049:] = 0.0

    # full IDFT matrix for the B side (alpha folded in)
    phase = (idx[:, None] * idx[None, :]) % D
    Br = tab_c[phase] * ALPHA
    Bni = tab_s[phase] * (-ALPHA)
    del phase

    Wb = np.asarray(W_base, np.float32)
    bb = np.asarray(b_base, np.float32)

    in_maps = []
    for m in range(NCORES):
        s = slice(m * SH, (m + 1) * SH)
        sk = slice(m * SHK, (m + 1) * SHK)
        in_maps.append({
            "xt": xT,
            "a_r": _tilemaj(Ar[:, sk]),
            "a_i": _tilemaj(Ai[:, sk]),
            "p_r": _tilemaj(Pr[:, s]),
            "p_i": _tilemaj(Pi[:, s]),
            "m_r": _tilemaj(Mr[:, s]),
            "m_i": _tilemaj(Mi[:, s]),
            "b_r": _tilemaj(Br[:, s]),
            "b_ni": _tilemaj(Bni[:, s]),
            "w_t": _tilemaj(np.ascontiguousarray(Wb[s, :].T)),
            "bias": bb[s].reshape(1, SH).astype(NP_BF16),
        })
    return in_maps


def kernel(x, W_base, b_base, c_re, c_im, mask_idx, _trace=False):
    if "nc" not in _CACHE:
        _CACHE["nc"] = _build_program()
    nc = _CACHE["nc"]
    in_maps = _host_prep(x, W_base, b_base, c_re, c_im, mask_idx)
    res = run_bass_kernel_spmd(nc, in_maps, list(range(NCORES)), trace=_trace)
    _CACHE["last"] = res
    y = np.concatenate([res.results[m]["y"] for m in range(NCORES)], axis=1)
    return y.reshape(2, 64, D).astype(np.float32)


# revision 8
# speedup vs baseline: 21.3286x; 21.3286x over previous
"""LFMA adapter kernel for 8 Trainium2 NeuronCores.

y = x @ W_base.T + b + alpha * x @ Re(ifft2(scatter(c)))      x:[2,64,4096]

With A = B = IDFT_4096 matrix (A[m,k] = exp(2pi i m k/4096)/4096) and F the
dense complex [4096,4096] scatter of c at mask_idx:

    y2 = alpha * Re( (x @ A) @ F @ B )

x is real, so u = x@A is Hermitian along k (u[N-k] = conj(u[k])): only
k = 0..2048 is needed (padded to KH=2176 = 17 tiles). The F contraction
folds into P[k] = F[k]+F[N-k], M[k] = F[k]-F[N-k]:
    vr = ur@Pr - ui@Mi,  vi = ur@Pi + ui@Mr
Sharding (8 cores): stage 1 u columns (272/core, all-gather uT), stage 2
P/M columns (512/core, all-gather vT), stage 3 output columns (512/core).
bf16 matmuls, fp32 PSUM accumulation; the y2 path is ~1e-3 of the output,
so its bf16 error is negligible; base matmul bf16 gives ~2e-3 absmax rel.
"""

import numpy as np
import ml_dtypes

import concourse.bass as bass
import concourse.mybir as mybir
import concourse.tile as tile
from concourse import bacc
from concourse.bass import ts
from concourse.bass_utils import run_bass_kernel_spmd
from concourse.masks import make_identity

BF16 = mybir.dt.bfloat16
F32 = mybir.dt.float32
F8 = mybir.dt.float8e4
NP_BF16 = ml_dtypes.bfloat16
NP_F8 = mybir.dt.np(mybir.dt.float8e4)
U_SCALE = 64.0   # u is ~0.0156 rms; x64 puts it in fp8e4m3's normal range

D = 4096          # d1 == d2
T = 128           # 2*64 flattened tokens
NCORES = 8
SH = D // NCORES  # 512 output columns per core
NT = D // 128     # 32 contraction tiles over d / l
KH = 2176         # half-spectrum k range 0..2048 padded to 17*128
KT = KH // 128    # 17 k tiles
SHK = KH // NCORES  # 272 u-columns per core
ALPHA = 16.0

_CACHE = {}


def _tilemaj(m):
    """[128*nt, n] -> tile-major [128, nt*n] bf16 (tile i at cols i*n:(i+1)*n)."""
    rows, n = m.shape
    nt = rows // 128
    return np.ascontiguousarray(
        m.reshape(nt, 128, n).transpose(1, 0, 2).reshape(128, nt * n)
    ).astype(NP_BF16)


def _build_program(reps=1):
    nc = bacc.Bacc("TRN2", target_bir_lowering=False, debug=False,
                   num_devices=NCORES)
    BIG = [128, NT * SH]
    PMS = [128, KT * SH]
    AS = [128, NT * SHK]

    xt = nc.dram_tensor("xt", [128, NT * 128], BF16, kind="ExternalInput")
    a_r = nc.dram_tensor("a_r", AS, BF16, kind="ExternalInput")
    a_i = nc.dram_tensor("a_i", AS, BF16, kind="ExternalInput")
    p_r = nc.dram_tensor("p_r", PMS, F8, kind="ExternalInput")
    p_i = nc.dram_tensor("p_i", PMS, F8, kind="ExternalInput")
    m_r = nc.dram_tensor("m_r", PMS, F8, kind="ExternalInput")
    m_i = nc.dram_tensor("m_i", PMS, F8, kind="ExternalInput")
    b_r = nc.dram_tensor("b_r", BIG, BF16, kind="ExternalInput")
    b_ni = nc.dram_tensor("b_ni", BIG, BF16, kind="ExternalInput")
    w_t = nc.dram_tensor("w_t", BIG, BF16, kind="ExternalInput")
    bias = nc.dram_tensor("bias", [1, SH], BF16, kind="ExternalInput")
    y_out = nc.dram_tensor("y", [T, SH], F32, kind="ExternalOutput")

    RG = [list(range(NCORES))]

    with tile.TileContext(nc) as tc:
        with (
            tc.tile_pool(name="dram", bufs=1, space="DRAM") as dramp,
            tc.tile_pool(name="const", bufs=1) as constp,
            tc.tile_pool(name="wpool", bufs=4) as wpool,
            tc.tile_pool(name="work", bufs=2) as work,
            tc.tile_pool(name="acc", bufs=1, space="PSUM") as accp,
            tc.tile_pool(name="trp", bufs=4, space="PSUM") as trp,
        ):
            ident = constp.tile([128, 128], BF16, name="ident")
            make_identity(nc, ident)
            ones = constp.tile([1, 128], BF16, name="ones")
            nc.vector.memset(ones, 1.0)
            bias_sb = constp.tile([1, SH], BF16, name="bias_sb")
            nc.sync.dma_start(bias_sb, bias[:])

            xt_sb = constp.tile([128, NT * 128], BF16, name="xt_sb")
            nc.sync.dma_start(xt_sb, xt[:])
            xt_v = xt_sb.rearrange("p (i c) -> p i c", i=NT)

            ut_all = constp.tile([128, KT * 256], BF16, name="ut_all")
            ut_f8 = constp.tile([128, KT * 256], F8, name="ut_f8")
            ut8_v = ut_f8.rearrange("p (i c) -> p i c", i=KT)
            vt_all = constp.tile([128, NT * 256], BF16, name="vt_all")
            ut_v = ut_all.rearrange("p (i c) -> p i c", i=KT)
            vt_v = vt_all.rearrange("p (i c) -> p i c", i=NT)

            for _rep in range(reps):
                ut_sh = dramp.tile([SHK, 2, 128], BF16, tag="ut_sh",
                                   name=f"ut_sh{_rep}")
                ut_ag = dramp.tile([KH, 2, 128], BF16, tag="ut_ag",
                                   name=f"ut_ag{_rep}", addr_space="Shared")
                vt_sh = dramp.tile([4, 128, 256], BF16, tag="vt_sh",
                                   name=f"vt_sh{_rep}")
                vt_ag = dramp.tile([NT, 128, 256], BF16, tag="vt_ag",
                                   name=f"vt_ag{_rep}", addr_space="Shared")

                def load_big(src, shape, nm, dt=BF16):
                    t = wpool.tile(shape, dt, tag="big", name=nm)
                    nc.gpsimd.dma_start(t, src[:])
                    return t

                ar_sb = load_big(a_r, AS, f"ar_sb{_rep}")
                ai_sb = load_big(a_i, AS, f"ai_sb{_rep}")
                pr_sb = load_big(p_r, PMS, f"pr_sb{_rep}", F8)
                mi_sb = load_big(m_i, PMS, f"mi_sb{_rep}", F8)
                pi_sb = load_big(p_i, PMS, f"pi_sb{_rep}", F8)
                mr_sb = load_big(m_r, PMS, f"mr_sb{_rep}", F8)

                # ---- stage 1: u = x @ A_shard   [128, 272] per component ----
                ps_ur = accp.tile([T, SHK], F32, tag="s1a", name=f"ps_ur{_rep}")
                ps_ui = accp.tile([T, SHK], F32, tag="s1b", name=f"ps_ui{_rep}")
                for i in range(NT):
                    nc.tensor.matmul(ps_ur, xt_v[:, i], ar_sb[:, ts(i, SHK)],
                                     start=(i == 0), stop=(i == NT - 1))
                for i in range(NT):
                    nc.tensor.matmul(ps_ui, xt_v[:, i], ai_sb[:, ts(i, SHK)],
                                     start=(i == 0), stop=(i == NT - 1))

                ur_sb = work.tile([T, SHK], BF16, tag="uv", name=f"ur{_rep}")
                nc.vector.tensor_copy(out=ur_sb, in_=ps_ur)
                ui_sb = work.tile([T, SHK], BF16, tag="uv", name=f"ui{_rep}")
                nc.vector.tensor_copy(out=ui_sb, in_=ps_ui)
                # transpose 272 = 2x128 + 16 into [k, comp, t] staging
                stg = work.tile([128, 3 * 256], BF16, tag="stg",
                                name=f"ustg{_rep}")
                for j in range(3):
                    rows = 128 if j < 2 else 16
                    for h, comp in enumerate((ur_sb, ui_sb)):
                        pt = trp.tile([128, 128], BF16, tag="tr",
                                      name=f"utp{_rep}_{j}_{h}")
                        nc.tensor.transpose(
                            pt[:rows, :], comp[:, j * 128: j * 128 + rows],
                            ident)
                        nc.vector.tensor_copy(
                            out=stg[:rows, j * 256 + h * 128:
                                    j * 256 + h * 128 + 128],
                            in_=pt[:rows, :])
                nc.sync.dma_start(
                    out=ut_sh[0:256].rearrange("(j p) h c -> p j h c", p=128),
                    in_=stg[:, 0:512].rearrange("p (j h c) -> p j h c",
                                                j=2, h=2))
                nc.sync.dma_start(
                    out=ut_sh[256:272],
                    in_=stg[0:16, 512:768].rearrange("p (h c) -> p h c", h=2))

                nc.gpsimd.collective_compute(
                    "AllGather", mybir.AluOpType.bypass,
                    ins=[ut_sh.opt()], outs=[ut_ag.opt()], replica_groups=RG)

                nc.sync.dma_start(
                    out=ut_all.rearrange("p (i c) -> p i c", i=KT),
                    in_=ut_ag.rearrange("(i p) h c -> p i (h c)", p=128))
                nc.vector.tensor_scalar_mul(ut_f8, ut_all, U_SCALE)

                # ---- stage 2: vr = ur@Pr - ui@Mi ; vi = ur@Pi + ui@Mr ----
                ps_a = accp.tile([T, SH], F32, tag="s1a", name=f"ps_a{_rep}")
                ps_b = accp.tile([T, SH], F32, tag="s1b", name=f"ps_b{_rep}")
                ps_c = accp.tile([T, SH], F32, tag="s2c", name=f"ps_c{_rep}")
                for i in range(KT):
                    nc.tensor.matmul(ps_a, ut8_v[:, i, 0:128],
                                     pr_sb[:, ts(i, SH)],
                                     start=(i == 0), stop=(i == KT - 1))
                for i in range(KT):
                    nc.tensor.matmul(ps_b, ut8_v[:, i, 128:256],
                                     mi_sb[:, ts(i, SH)],
                                     start=(i == 0), stop=(i == KT - 1))
                for i in range(KT):
                    nc.tensor.matmul(ps_c, ut8_v[:, i, 0:128],
                                     pi_sb[:, ts(i, SH)],
                                     start=(i == 0), stop=False)
                for i in range(KT):
                    nc.tensor.matmul(ps_c, ut8_v[:, i, 128:256],
                                     mr_sb[:, ts(i, SH)],
                                     start=False, stop=(i == KT - 1))
                # descale the fp8 u amplification (PSUM holds U_SCALE * v)
                tmp_a = work.tile([T, SH], F32, tag="tmpf", name=f"tmpa{_rep}")
                nc.vector.tensor_scalar_mul(tmp_a, ps_a, 1.0 / U_SCALE)
                tmp_b = work.tile([T, SH], F32, tag="tmpf", name=f"tmpb{_rep}")
                nc.vector.tensor_scalar_mul(tmp_b, ps_b, 1.0 / U_SCALE)
                vr_sb = work.tile([T, SH], BF16, tag="uv2", name=f"vr{_rep}")
                nc.vector.tensor_sub(out=vr_sb, in0=tmp_a, in1=tmp_b)
                vi_sb = work.tile([T, SH], BF16, tag="uv2", name=f"vi{_rep}")
                nc.vector.tensor_scalar_mul(vi_sb, ps_c, 1.0 / U_SCALE)

                br_sb = load_big(b_r, BIG, f"br_sb{_rep}")
                bni_sb = load_big(b_ni, BIG, f"bni_sb{_rep}")

                stg2 = work.tile([128, 4 * 256], BF16, tag="stg",
                                 name=f"vstg{_rep}")
                for j in range(4):
                    for h, comp in enumerate((vr_sb, vi_sb)):
                        pt = trp.tile([128, 128], BF16, tag="tr",
                                      name=f"vtp{_rep}_{j}_{h}")
                        nc.tensor.transpose(pt, comp[:, ts(j, 128)], ident)
                        nc.vector.tensor_copy(
                            out=stg2[:, j * 256 + h * 128:
                                     j * 256 + (h + 1) * 128],
                            in_=pt)
                nc.sync.dma_start(
                    out=vt_sh.rearrange("j p c -> p j c"),
                    in_=stg2.rearrange("p (j c) -> p j c", j=4))

                nc.gpsimd.collective_compute(
                    "AllGather", mybir.AluOpType.bypass,
                    ins=[vt_sh.opt()], outs=[vt_ag.opt()], replica_groups=RG)

                nc.sync.dma_start(
                    out=vt_all.rearrange("p (i c) -> p i c", i=NT),
                    in_=vt_ag.rearrange("i p c -> p i c"))

                wt_sb = load_big(w_t, BIG, f"wt_sb{_rep}")

                # ---- stage 3: y = vTr.T@(aBr) + vTi.T@(-aBi) + x@WbT + b ----
                ps_y = accp.tile([T, SH], F32, tag="s3", name=f"ps_y{_rep}")
                for i in range(NT):
                    nc.tensor.matmul(ps_y, vt_v[:, i, 0:128],
                                     br_sb[:, ts(i, SH)],
                                     start=(i == 0), stop=False)
                for i in range(NT):
                    nc.tensor.matmul(ps_y, vt_v[:, i, 128:256],
                                     bni_sb[:, ts(i, SH)],
                                     start=False, stop=False)
                for i in range(NT):
                    nc.tensor.matmul(ps_y, xt_v[:, i], wt_sb[:, ts(i, SH)],
                                     start=False, stop=False)
                nc.tensor.matmul(ps_y, ones, bias_sb, start=False, stop=True)

                y_sb = work.tile([T, SH], F32, tag="ysb", name=f"y_sb{_rep}")
                nc.vector.tensor_copy(out=y_sb, in_=ps_y)
                nc.sync.dma_start(out=y_out[:], in_=y_sb)

    nc.compile()
    return nc


def _host_prep(x, W_base, b_base, c_re, c_im, mask_idx):
    xf = np.asarray(x, np.float32).reshape(T, D)
    xT = _tilemaj(np.ascontiguousarray(xf.T))

    # exact-phase IDFT matrix via cos/sin table lookup
    idx = np.arange(D, dtype=np.int64)
    tab_c = (np.cos(2 * np.pi * np.arange(D) / D) / D).astype(np.float32)
    tab_s = (np.sin(2 * np.pi * np.arange(D) / D) / D).astype(np.float32)
    ph_half = (idx[:, None] * idx[None, :KH]) % D     # [4096, 2176]
    Ar = tab_c[ph_half]
    Ai = tab_s[ph_half]
    Ar[:, 2049:] = 0.0
    Ai[:, 2049:] = 0.0
    del ph_half

    Fr = np.zeros(D * D, np.float32)
    Fi = np.zeros(D * D, np.float32)
    mi = np.asarray(mask_idx, np.int64)
    Fr[mi] = np.asarray(c_re, np.float32)
    Fi[mi] = np.asarray(c_im, np.float32)
    Fr = Fr.reshape(D, D)
    Fi = Fi.reshape(D, D)
    rev = (-np.arange(KH)) % D
    Pr = Fr[:KH] + Fr[rev]
    Pi = Fi[:KH] + Fi[rev]
    Mr = Fr[:KH] - Fr[rev]
    Mi = Fi[:KH] - Fi[rev]
    for M_ in (Pr, Pi, Mr, Mi):
        M_[0] *= 0.5
        M_[2048] *= 0.5
        M_[2049:] = 0.0

    # full IDFT matrix for the B side (alpha folded in)
    phase = (idx[:, None] * idx[None, :]) % D
    Br = tab_c[phase] * ALPHA
    Bni = tab_s[phase] * (-ALPHA)
    del phase

    Wb = np.asarray(W_base, np.float32)
    bb = np.asarray(b_base, np.float32)

    in_maps = []
    for m in range(NCORES):
        s = slice(m * SH, (m + 1) * SH)
        sk = slice(m * SHK, (m + 1) * SHK)
        in_maps.append({
            "xt": xT,
            "a_r": _tilemaj(Ar[:, sk]),
            "a_i": _tilemaj(Ai[:, sk]),
            "p_r": _tilemaj(Pr[:, s]).astype(NP_F8),
            "p_i": _tilemaj(Pi[:, s]).astype(NP_F8),
            "m_r": _tilemaj(Mr[:, s]).astype(NP_F8),
            "m_i": _tilemaj(Mi[:, s]).astype(NP_F8),
            "b_r": _tilemaj(Br[:, s]),
            "b_ni": _tilemaj(Bni[:, s]),
            "w_t": _tilemaj(np.ascontiguousarray(Wb[s, :].T)),
            "bias": bb[s].reshape(1, SH).astype(NP_BF16),
        })
    return in_maps


def kernel(x, W_base, b_base, c_re, c_im, mask_idx, _trace=False):
    if "nc" not in _CACHE:
        _CACHE["nc"] = _build_program()
    nc = _CACHE["nc"]
    in_maps = _host_prep(x, W_base, b_base, c_re, c_im, mask_idx)
    res = run_bass_kernel_spmd(nc, in_maps, list(range(NCORES)), trace=_trace)
    _CACHE["last"] = res
    y = np.concatenate([res.results[m]["y"] for m in range(NCORES)], axis=1)
    return y.reshape(2, 64, D).astype(np.float32)
